# revision 1
# baseline (speedup 1.0000x reference)
"""BallQuery kernel for Trainium2 (Bass/Tile), data-parallel over batch on 8 cores.

Problem: xyz (8, 16384, 3) points, new_xyz (8, 1024, 3) query centers.
For each query, return the first NSAMPLE=32 point indices (ascending) with
squared distance < RADIUS^2; pad with the first found index; all-sentinel
(N+1) rows when no point is in the ball.  Output int32 (8, 1024, 32).

Algorithm per core (one batch):
  - PE matmul (K=4): psum = |x|^2 + sum_d (-2*x_d)*q_d  == |x|^2 - 2 q.x
    (weights = queries, moving = points; PE quadrant tiling packs the
    4 x-chunk groups at partition bases 0/32/64/96)
  - d2 = psum + |q|^2 (per-partition scalar), mask = d2 < R^2.
  - v = mask * (N - n): nonzero exactly at in-ball points; value encodes
    the position such that bigger value == smaller index.
  - 4 rounds of DVE max8 + match_replace extract the 32 largest v per query
    row in descending order == the 32 smallest in-ball indices ascending.
  - idx = N - v, with reference padding/sentinel semantics applied.

Structural constraint honored throughout: a DMA instruction supports only
ONE semaphore wait, so every DMA depends on at most one producer; engine
instructions keep <=3 waits.
"""

import os
import numpy as np

import concourse.bass as bass
import concourse.bacc as bacc
import concourse.mybir as mybir
import concourse.tile as tile
from concourse import bass_utils

F32 = mybir.dt.float32
I32 = mybir.dt.int32

N = 16384  # points per batch
M = 1024  # queries per batch
B = 8  # batches == cores
NS = 32  # samples per query
R2 = 0.15 * 0.15
MT = 128  # queries per m-tile
N_MT = M // MT  # 8
CH = 2048  # psum-group width (4 matmuls of 512)
N_CH = N // CH  # 8
MM = 512  # single matmul free dim
N_SLOT = N // (4 * MM)  # 8 free slots per quadrant group
SENTINEL = float(N + 1)


def build(nc: bass.Bass, repeat: int = 1):
    xyz_t = nc.dram_tensor("xyz", [N, 3], F32, kind="ExternalInput")
    q_t = nc.dram_tensor("new_xyz", [M, 3], F32, kind="ExternalInput")
    iot_t = nc.dram_tensor("iota_rev", [128, N], F32, kind="ExternalInput")
    out_t = nc.dram_tensor("out", [M, NS], I32, kind="ExternalOutput")
    scrb = nc.dram_tensor("scrb", [N], F32)  # -0.5*|x|^2 staging

    xyz_ap = xyz_t.ap()
    q_ap = q_t.ap()
    out_ap = out_t.ap()

    mul = mybir.AluOpType.mult
    add = mybir.AluOpType.add

    with tile.TileContext(nc) as tc:
        import contextlib

        with contextlib.ExitStack() as ctx:
            const_pool = ctx.enter_context(tc.tile_pool(name="const", bufs=1))
            prep_pool = ctx.enter_context(tc.tile_pool(name="prep", bufs=1))
            v_pool = ctx.enter_context(tc.tile_pool(name="v", bufs=3))
            psum_pool = ctx.enter_context(
                tc.tile_pool(name="psum", bufs=2, space="PSUM")
            )
            small_pool = ctx.enter_context(tc.tile_pool(name="small", bufs=3))

            # ---------------- one-time prep ----------------
            # -0.5*|x|^2 in wrapped layout, staged to DRAM in linear order
            xyzw = const_pool.tile([128, N // 128 * 3], F32)  # [128, 384]
            nc.sync.dma_start(xyzw[:], xyz_ap.rearrange("(p a) d -> p (a d)", p=128))
            xyzw3 = xyzw[:].rearrange("p (a d) -> p a d", d=3)  # [128, 128, 3]
            sq = prep_pool.tile([128, 128], F32)
            t2 = prep_pool.tile([128, 128], F32)
            nc.vector.tensor_tensor(sq[:], xyzw3[:, :, 0], xyzw3[:, :, 0], mul)
            nc.vector.tensor_tensor(t2[:], xyzw3[:, :, 1], xyzw3[:, :, 1], mul)
            nc.vector.tensor_tensor(sq[:], sq[:], t2[:], add)
            nc.vector.tensor_tensor(t2[:], xyzw3[:, :, 2], xyzw3[:, :, 2], mul)
            nc.vector.tensor_tensor(sq[:], sq[:], t2[:], add)
            nc.vector.tensor_scalar(sq[:], sq[:], -0.5, None, op0=mul)
            nc.sync.dma_start(scrb.ap(), sq[:])

            # A = |q|^2 in transposed layout At[p, a] = A[a*128+p], computed
            # from direct transposed loads of the query coords (no roundtrip)
            qtw = const_pool.tile([128, 3 * N_MT], F32)
            qtw3 = qtw[:].rearrange("p (d a) -> p d a", d=3)
            qT = q_ap.rearrange("(a p) d -> d p a", p=128)  # [3, 128, 8]
            for d in range(3):
                nc.sync.dma_start(qtw3[:, d, :], qT[d])
            At = const_pool.tile([128, N_MT], F32)
            tA = prep_pool.tile([128, N_MT], F32)
            nc.vector.tensor_tensor(At[:], qtw3[:, 0, :], qtw3[:, 0, :], mul)
            nc.vector.tensor_tensor(tA[:], qtw3[:, 1, :], qtw3[:, 1, :], mul)
            nc.vector.tensor_tensor(At[:], At[:], tA[:], add)
            nc.vector.tensor_tensor(tA[:], qtw3[:, 2, :], qtw3[:, 2, :], mul)
            nc.vector.tensor_tensor(At[:], At[:], tA[:], add)

            # qr (lhsT): per quadrant base 32p, row +0 = ones, rows +1..3 = q_d
            qr = const_pool.tile([100, M], F32)
            qrT = q_ap.rearrange("m d -> d m")  # [3, 1024] strided
            for par in range(4):
                b = 32 * par
                nc.vector.memset(qr[b : b + 1, :], 1.0)
                nc.sync.dma_start(qr[b + 1 : b + 4, :], qrT)

            # xr (rhs): per quadrant base 32p: row +0 = -0.5|x|^2, rows +1..3 =
            # x_d for chunks c = 4s+par; then one consolidating *(-2) so the
            # matmul depends on a single producer.  (-2)*(-0.5|x|^2) = |x|^2.
            xr = const_pool.tile([100, N_SLOT * MM], F32)
            xT = xyz_ap.rearrange("(s q w) d -> q d s w", q=4, w=MM)  # [4,3,8,512]
            bT = scrb.ap().rearrange("(s q w) -> q s w", q=4, w=MM)  # [4,8,512]
            for par in range(4):
                b = 32 * par
                for d in range(3):
                    nc.sync.dma_start(
                        xr[b + 1 + d : b + 2 + d, :].rearrange(
                            "k (s w) -> k s w", w=MM
                        ),
                        xT[par : par + 1, d],
                    )
                nc.sync.dma_start(
                    xr[b : b + 1, :].rearrange("k (s w) -> k s w", w=MM),
                    bT[par : par + 1],
                )
                nc.scalar.mul(xr[b : b + 4, :], xr[b : b + 4, :], -2.0)

            # iotaR[:, j] = N - j (host-provided constant input)
            iotaR = const_pool.tile([128, N], F32)
            nc.sync.dma_start(iotaR[:], iot_t.ap())

            w_pool = ctx.enter_context(tc.tile_pool(name="w", bufs=2))

            # ---------------- main loop over m-tiles ----------------
            for mt_rep in range(N_MT * repeat):
                mt = mt_rep % N_MT
                v = v_pool.tile([128, N], mybir.dt.uint16)
                for c4 in range(N_CH):
                    pt = psum_pool.tile([128, CH], F32)
                    for cc in range(CH // MM):
                        ch = c4 * (CH // MM) + cc
                        par, slot = ch % 4, ch // 4
                        b = 32 * par
                        nc.tensor.matmul(
                            pt[:, cc * MM : (cc + 1) * MM],
                            qr[b : b + 4, mt * MT : (mt + 1) * MT],
                            xr[b : b + 4, slot * MM : (slot + 1) * MM],
                            start=True,
                            stop=True,
                            tile_position=(b, 0),
                        )
                    # ACT: w = psum + A (same f32 association as before)
                    w = w_pool.tile([128, CH], F32)
                    nc.scalar.activation(
                        w[:], pt[:], mybir.ActivationFunctionType.Identity,
                        bias=At[:, mt : mt + 1], scale=1.0,
                    )
                    # GPSIMD: w = (w < R2) * (N - n) staged in f32 in place
                    # (Pool integer TT unsupported), then ACT copy-converts
                    # to the uint16 v plane.
                    nc.gpsimd.tensor_scalar(
                        w[:], w[:], float(R2), None, op0=mybir.AluOpType.is_lt
                    )
                    nc.gpsimd.tensor_tensor(
                        w[:], w[:], iotaR[:, c4 * CH : (c4 + 1) * CH], mul
                    )
                    nc.scalar.copy(v[:, c4 * CH : (c4 + 1) * CH], w[:])

                # extract top-32 (descending v == ascending index).
                # Round 1 is split into two half-row max8s + a tiny merge so
                # it can start as soon as the first half of v is written.
                vals = small_pool.tile([128, NS], mybir.dt.uint16)
                h16 = small_pool.tile([128, 16], mybir.dt.uint16)
                nc.vector.max(h16[:, 0:8], v[:, : N // 2])
                nc.vector.max(h16[:, 8:16], v[:, N // 2 :])
                nc.vector.max(vals[:, 0:8], h16[:])
                nc.vector.match_replace(
                    out=v[:], in_to_replace=vals[:, 0:8], in_values=v[:],
                    imm_value=0.0,
                )
                for r in range(1, 4):
                    nc.vector.max(vals[:, 8 * r : 8 * r + 8], v[:])
                    if r < 3:
                        nc.vector.match_replace(
                            out=v[:],
                            in_to_replace=vals[:, 8 * r : 8 * r + 8],
                            in_values=v[:],
                            imm_value=0.0,
                        )

                # idx = N - v ; pad empties with first column; all-empty -> N+1
                idxf = small_pool.tile([128, NS], F32)
                nc.vector.tensor_scalar(
                    idxf[:], vals[:], -1.0, float(N), op0=mul, op1=add
                )
                inv = small_pool.tile([128, NS], mybir.dt.uint32)
                nc.vector.tensor_scalar(
                    inv[:], vals[:], 0.0, None, op0=mybir.AluOpType.is_equal
                )
                nc.vector.copy_predicated(
                    idxf[:], inv[:], idxf[:, 0:1].to_broadcast([128, NS])
                )
                sent = small_pool.tile([128, 1], F32)
                nc.vector.memset(sent[:], SENTINEL)
                nc.vector.copy_predicated(
                    idxf[:],
                    inv[:, 0:1].to_broadcast([128, NS]),
                    sent[:].to_broadcast([128, NS]),
                )
                outt = small_pool.tile([128, NS], I32)
                nc.vector.tensor_copy(outt[:], idxf[:])
                nc.sync.dma_start(out_ap[mt * MT : (mt + 1) * MT, :], outt[:])

    return nc


_NC_CACHE = {}
LAST_RESULT = None
TRACE = bool(int(os.environ.get("BALLQ_TRACE", "0")))


def _get_nc(repeat: int = 1):
    if repeat not in _NC_CACHE:
        nc = bacc.Bacc("TRN2", target_bir_lowering=False, debug=False)
        build(nc, repeat)
        nc.compile()
        _NC_CACHE[repeat] = nc
    return _NC_CACHE[repeat]


def _iota_rev() -> np.ndarray:
    return np.broadcast_to(
        (N - np.arange(N, dtype=np.float32))[None, :], (128, N)
    ).copy()


def kernel(**inputs) -> np.ndarray:
    global LAST_RESULT
    xyz = np.ascontiguousarray(np.asarray(inputs["xyz"], dtype=np.float32))
    new_xyz = np.ascontiguousarray(np.asarray(inputs["new_xyz"], dtype=np.float32))
    assert xyz.shape == (B, N, 3) and new_xyz.shape == (B, M, 3)

    nc = _get_nc(int(os.environ.get("BALLQ_REPEAT", "1")))
    iota_rev = _iota_rev()
    in_maps = [
        {"xyz": xyz[b], "new_xyz": new_xyz[b], "iota_rev": iota_rev}
        for b in range(B)
    ]
    res = bass_utils.run_bass_kernel_spmd(nc, in_maps, list(range(B)), trace=TRACE)
    LAST_RESULT = res
    out = np.stack([res.results[b]["out"] for b in range(B)], axis=0)
    return out.astype(np.int32)

